# revision 88
# baseline (speedup 1.0000x reference)
"""Trainium2 Bass kernel: TextCNN (conv k=3/4/5 over [B,1,S,E] + relu +
global max-pool + FC + log_softmax), data-parallel over batch on 8 cores.

Conv runs as fp8 e4m3 DoubleRow matmuls: each contracts 256 rows (the
two 128-row E-chunks paired along the DR j-dim), so a 4-batch group
needs 15 matmuls instead of 30, streaming at the PE's doubled fp8
rate. Weights are pre-scaled by 2^7 into e4m3's normal range; the
scale divides out for free via the ReLU activation's `scale` operand.
x is stored s-major (col n = s*G+b) so ONE resident copy serves all
tap shifts as contiguous 4-col-step slices — the dual-fp8 ISA allows
only [pair, cols] rhs patterns with 16B-aligned steps, which rules
out b-major windowed views. Each tap's e[256:300] tail is host-packed
two-rows-per-partition into one extra DR matmul per branch. PSUM is
[112(f pad16), S, G]; the max-pool reduce reads it through a
transposed strided AP. The FC runs in bf16 with features stationary,
emitting [16 batch, 5 class] transposed slices: the class sum then
falls out of the Exp activation's accum_out and the log-subtract is a
single Identity-with-bias, so each slice's log_softmax is a short
all-Scalar chain (the kernel emits ln(sum)-logits; the host gather
negates). FC + softmax slices pipeline through the conv stream, so
only the last slice's chain sits on the kernel tail.

Schedule notes: the PE stream is pinned with nosync chain deps (the
tile scheduler otherwise hoists matmuls above their wait carriers and
breaks the single-wait-slot TPB encoding); every cross-queue wait is
pre-satisfied on a spare matmul of the previous group. The two DMA
rings ramp with their own activity, so transfers alternate across
them in need-order, with each group's small main-plane block split
out ahead of its tails, and groups 0/1's mains interleaved before
either group's tails. Junk prewarm matmuls bridge the DMA ramp so the
PE clock gate is up when real work starts. The kernel-tail drain is
split per semaphore proc (same single-wait-slot limit).

Self-contained: hardcodes shapes/sharding; only imports the container
toolchain at /opt/trn_rl_repo.
"""

import sys

import ml_dtypes
import numpy as np

sys.path.insert(0, "/opt/trn_rl_repo")

import concourse.bass as bass  # noqa: E402
import concourse.tile as tile  # noqa: E402
from concourse import mybir  # noqa: E402
from concourse.bass_utils import run_bass_kernel_spmd  # noqa: E402
from concourse.tile import add_dep_helper  # noqa: E402
from concourse.vector_clock import ScopedClock, VectorClock  # noqa: E402

B, S, E = 512, 128, 300
NF = 100
NCLS = 5
NCORES = 8
BPC = B // NCORES  # 64 batch elems per core
G = 4  # batch elems per matmul group (4*128 = 512 psum cols)
NG = BPC // G  # 16 groups
PAD = 2
SP = S + 2 * PAD  # 132 padded seq length
KS = (3, 4, 5)
SOUT = {3: S - 2, 4: S - 1, 5: S}  # valid conv output positions per branch
SMM = S  # uniform matmul col window; invalid tail cols excluded by reduce
E2 = 256  # main contraction rows (two 128 chunks paired along DR j)
E2N = E - E2  # 44 tail rows per tap
WS = 128.0  # host weight scale into e4m3 range; divided out in ReLU
NFP = 112  # M padded to a 16B multiple (dual-fp8 LDW stride alignment)
# main taps in matmul order: (k, i, off); off = (5-k)+i is the xpad shift
MAINTAPS = [(k, i, 5 - k + i) for k in KS for i in range(k)]
NTAP = len(MAINTAPS)  # 12
NW = NTAP + 3  # + one packed tail per branch
WCOLS = NW * 2 * NFP  # 3360 fp8 weight cols
NMM = SMM * G  # 512 moving cols per matmul, s-major (n = s*G + b)
SPW = SP * G  # 528-col s-major main plane; shift o = slice [4o : 4o+512]
GCOLS = 2 * (SPW + 3 * NMM)  # 4128 fp8 x cols per group (j-major)
TOTB = WCOLS + NG * GCOLS  # start of the last group's b-major region
BCOLS = 8 * 2 * NMM  # 5 shifted mains + 3 tails, [128, 2, 512] each
TOTC = TOTB + BCOLS
AUXW = 3 + 3 * NCLS
NPRE = 5  # prewarm matmuls bridging DMA ramp + PE clock ramp

_f32 = mybir.dt.float32
_bf16 = mybir.dt.bfloat16
_fp8 = mybir.dt.float8e4
_DR = mybir.MatmulPerfMode.DoubleRow

_built = None


def _ins(i):
    return i.ins if hasattr(i, "ins") else i


def _dep(from_inst, to_inst, reason, sync=True):
    add_dep_helper(_ins(from_inst), _ins(to_inst), sync=sync, reason=reason)


class _SplitDrainTC(tile.TileContext):
    """TileContext whose kernel-tail drain is split into one drain per
    semaphore proc: the stock single drain carries one wait per used proc,
    which overflows the CTRL_NO encoding's wait slots on this toolchain."""

    def _drain_and_barrier(self, tick_clock, wait_clock):
        gc = tick_clock.global_clock
        ticks = eval(str(gc).replace("VectorClock", ""))
        for idx, tick in enumerate(ticks):
            if tick > 0:
                sub = VectorClock()
                sub.require_at_least(idx, tick)
                d = self.nc.sync.drain()
                wait_clock.add_sem_waits(d.ins, ScopedClock({None: sub}))
        self.nc.all_engine_barrier()
        assert self.sems is not None
        popped = self.nc._tile_sem_poison_stack.pop()
        assert popped is self._sem_poison
        self.nc.clear_and_free_semaphores(list(self.sems.allocated().values()))
        self.nc.all_engine_barrier()


def _build():
    nc = bass.Bass()
    xq = nc.declare_dram_parameter("xq", [128, TOTC], _fp8, isOutput=False)
    aux = nc.declare_dram_parameter("aux", [NF + 1, AUXW], _f32, isOutput=False)
    auxfc = nc.declare_dram_parameter(
        "auxfc", [NF + 1, 3 * NCLS], _bf16, isOutput=False
    )
    out = nc.declare_dram_parameter("out", [BPC, NCLS], _f32, isOutput=True)

    act = mybir.ActivationFunctionType

    with _SplitDrainTC(nc) as tc:
        with (
            tc.tile_pool(name="consts", bufs=1) as consts,
            tc.tile_pool(name="xin", bufs=NG) as xin,
            tc.tile_pool(name="small", bufs=4) as small,
            tc.tile_pool(name="feat", bufs=1) as featp,
            tc.tile_pool(name="psum", bufs=2, space="PSUM") as psum,
            tc.tile_pool(name="psfc", bufs=1, space="PSUM") as psfc,
        ):
            pescr = psfc.tile([128, 512], _f32, tag="pescr")
            junk = small.tile([128, 2, 512], _fp8, tag="junk")
            # memset on gpsimd: its queue clears the preamble ~1.5us before
            # the Vector queue does, so the PE prewarm can start that early
            nc.gpsimd.memset(junk[:], 0.25)

            wtile = consts.tile([128, NW, 2, NFP], _fp8, tag="w", name="w")
            wdma = nc.sync.dma_start(
                out=wtile[:],
                in_=xq[:, :WCOLS].rearrange("p (t j f) -> p t j f", t=NW, j=2),
            )

            xtiles = {}
            xmdmas = {}
            xtdmas = {}

            def make_xm(g, eng):
                t = xin.tile([128, 2, SPW + 3 * NMM], _fp8, tag="x", name=f"x_{g}")
                base = WCOLS + g * GCOLS
                # main plane first as its own small DMA: it unblocks the
                # group's main taps ~4x sooner than the full-group transfer
                # (the DMA engines run slow while ramping at kernel start)
                xmdmas[g] = eng.dma_start(
                    out=t[:, :, :SPW],
                    in_=xq[:, base : base + 2 * SPW].rearrange(
                        "p (j n) -> p j n", j=2
                    ),
                )
                xtiles[g] = t
                return t

            def make_xt(g, eng):
                t = xtiles[g]
                base = WCOLS + g * GCOLS
                xtdmas[g] = eng.dma_start(
                    out=t[:, :, SPW:],
                    in_=xq[:, base + 2 * SPW : base + GCOLS].rearrange(
                        "p (j n) -> p j n", j=2
                    ),
                )

            def make_x(g, eng):
                make_xm(g, eng)
                make_xt(g, eng)
                return xtiles[g]

            # two DMA rings share HBM bandwidth and each ramps with its own
            # activity: alternate groups across them in need-order so the
            # earliest pieces land first (spreading thinner across more
            # rings keeps them all cold — measured slower); the tiny aux
            # transfers slip between x0's main and tail
            auxt = consts.tile([NF + 1, AUXW], _f32, tag="aux", name="aux")
            auxfct = consts.tile([NF + 1, 3 * NCLS], _bf16, tag="auxfc", name="auxfc")
            make_xm(0, nc.gpsimd)
            aux_dma = nc.gpsimd.dma_start(out=auxt[:], in_=aux[:, :])
            auxfc_dma = nc.gpsimd.dma_start(out=auxfct[:], in_=auxfc[:, :])
            make_xt(0, nc.gpsimd)
            make_x(1, nc.sync)
            for g in range(2, NG - 1):
                make_x(g, nc.gpsimd if g % 2 == 0 else nc.sync)
            # last group rides a b-major layout (batch-contiguous blocks per
            # shift) so its max-pool reduces read PSUM contiguously — the
            # final reduce sits on the kernel's critical tail
            xb15 = xin.tile([128, 8, 2, NMM], _fp8, tag="xb15", name="xb15")
            xmdmas[NG - 1] = nc.sync.dma_start(
                out=xb15[:],
                in_=xq[:, TOTB:TOTC].rearrange("p (m j n) -> p m j n", m=8, j=2),
            )
            xtdmas[NG - 1] = None

            # prewarm: fp8 DR junk matmuls bridge the DMA ramp so the PE
            # clock is up when the real stream starts; last one fences wdma.
            # sync=False edges pin the scheduler to this PE order — without
            # them it hoists later matmuls above the wait-carrying ones and
            # the single-wait-slot budget breaks.
            last_pe = None
            for p in range(NPRE):
                pw = nc.tensor.matmul(
                    pescr[:, :],
                    junk[:, :, :128],
                    junk[:, :, :],
                    start=True,
                    stop=True,
                    perf_mode=_DR,
                )
                if last_pe is not None:
                    _dep(pw, last_pe, "pe chain", sync=False)
                last_pe = pw
                if p == NPRE - 1:
                    _dep(pw, wdma, "w loaded")

            ascratch = small.tile([1, 1], _f32, tag="ascratch")
            feats = [
                featp.tile([NF, BPC], _f32, tag=f"feat{kk}", name=f"feat{kk}")
                for kk in range(3)
            ]
            featr = [
                featp.tile(
                    [NF + (1 if kk == 2 else 0), BPC],
                    _bf16,
                    tag=f"featr{kk}",
                    name=f"featr{kk}",
                )
                for kk in range(3)
            ]
            nc.vector.memset(featr[2][:], 1.0)

            # FC output transposed: [16 batch, 5 class] per slice, so the
            # class-sum falls out of the Exp activation's accum_out and the
            # log-subtract is one per-partition tensor_scalar — no softmax
            # matmuls, two fewer engine hops on the kernel tail
            plT = psfc.tile([16, 4 * NCLS], _f32, tag="plT")
            afence = nc.scalar.memzero(ascratch[:])
            _dep(afence, aux_dma, "act waits aux")
            # touch Exp/Ln tables now so the tail doesn't pay cold loads
            nc.scalar.activation(ascratch[:], ascratch[:], act.Exp)
            nc.scalar.activation(ascratch[:], ascratch[:], act.Ln)

            gmms = {}  # group -> list of its 15 conv matmuls
            greds = {}  # group -> last reduce_max
            pstiles = {}  # group -> psum tiles by branch
            # per-slice scratch: distinct tiles so the sliced softmax stages
            # carry no same-engine WAW completion waits
            expTs = [
                small.tile([16, NCLS], _f32, tag=f"expT{s}", name=f"expT{s}")
                for s in range(4)
            ]
            sums = [
                small.tile([16, 1], _f32, tag=f"sum{s}", name=f"sum{s}")
                for s in range(4)
            ]
            lnss = [
                small.tile([16, 1], _f32, tag=f"lns{s}", name=f"lns{s}")
                for s in range(4)
            ]
            ots = [
                small.tile([16, NCLS], _f32, tag=f"ot{s}", name=f"ot{s}")
                for s in range(4)
            ]
            oscrs = [
                small.tile([1, 1], _f32, tag=f"oscr{s}", name=f"oscr{s}")
                for s in range(4)
            ]

            def _mm(g, *args, **kw):
                nonlocal last_pe
                m = nc.tensor.matmul(*args, **kw)
                _dep(m, last_pe, "pe chain", sync=False)
                last_pe = m
                if g is not None:
                    gmms[g].append(m)
                return m

            def emit_mains(g, seq=False):
                gmms[g] = []
                pstiles[g] = {}
                bm = g == NG - 1  # b-major layout for the last group
                xt = None if bm else xtiles[g]
                ti = 0
                for kk, k in enumerate(KS):
                    shape = [NFP, G, SMM] if bm else [NFP, SMM, G]
                    ps = psum.tile(shape, _f32, tag=f"y{k}", name=f"y{k}_{g}")
                    pstiles[g][kk] = ps
                    for i in range(k):
                        off = 5 - k + i
                        _mm(
                            g,
                            ps[:, :, :],
                            wtile[:, ti, :, :],
                            xb15[:, off, :, :]
                            if bm
                            else xt[:, :, 4 * off : 4 * off + NMM],
                            start=(i == 0),
                            stop=False,
                            perf_mode=_DR,
                        )
                        ti += 1
                    if seq:
                        emit_tail(g, kk)

            def emit_tail(g, kk):
                k = KS[kk]
                bm = g == NG - 1
                ps = pstiles[g][kk]
                _mm(
                    g,
                    ps[:, :, :],
                    wtile[:, NTAP + kk, :, :],
                    xb15[:, 5 + kk, :, :]
                    if bm
                    else xtiles[g][:, :, SPW + kk * NMM : SPW + (kk + 1) * NMM],
                    start=False,
                    stop=True,
                    perf_mode=_DR,
                )
                red = nc.vector.reduce_max(
                    feats[kk][:, g * G : (g + 1) * G],
                    ps[:NF, :, : SOUT[k]]
                    if bm
                    else ps[:NF, : SOUT[k], :].transpose([0, 2, 1]),
                    axis=mybir.AxisListType.X,
                )
                r = nc.scalar.activation(
                    featr[kk][:NF, g * G : (g + 1) * G],
                    feats[kk][:, g * G : (g + 1) * G],
                    act.Relu,
                    bias=auxt[:NF, kk : kk + 1],
                    scale=1.0 / WS,
                )
                _dep(r, afence, "act fence", sync=False)
                greds[g] = red

            def emit_tails(g):
                for kk in range(3):
                    emit_tail(g, kk)

            fc_last = {}

            def _fc(s):
                for kk in range(3):
                    krows = NF + (1 if kk == 2 else 0)
                    wsl = auxfct[:krows, NCLS * kk : NCLS * (kk + 1)]
                    fc_last[s] = _mm(
                        None,
                        plT[:, NCLS * s : NCLS * (s + 1)],
                        featr[kk][:krows, 16 * s : 16 * (s + 1)],
                        wsl,
                        start=(kk == 0),
                        stop=(kk == 2),
                    )

            # log_softmax pipelined per 16-batch FC slice: each stage spread
            # a group apart so every instruction's single cross-engine wait
            # is satisfied by the time its engine reaches it; only slice 3's
            # chain remains on the kernel tail.
            sl = lambda s: slice(NCLS * s, NCLS * (s + 1))

            def smx_exp(s):
                # whole chain on the Scalar queue: the class sum falls out of
                # Exp's accum_out, then -ln(sum) via Ln + negating Copy, and
                # the log-subtract is Identity with a per-partition bias — no
                # cross-engine hops after the FC matmul
                nc.scalar.activation(
                    expTs[s][:], plT[:, sl(s)], act.Exp, accum_out=sums[s][:]
                )
                nc.scalar.activation(lnss[s][:], sums[s][:], act.Ln)

            def smx_out(s):
                # ln(sum) - logits; the host gather negates (free there)
                cp = nc.scalar.activation(
                    ots[s][:],
                    plT[:, sl(s)],
                    act.Identity,
                    bias=lnss[s][:],
                    scale=-1.0,
                )
                f = nc.scalar.memzero(oscrs[s][:])
                _dep(f, cp, "ot slice ready")
                nc.scalar.dma_start(
                    out=out[16 * s : 16 * (s + 1), :], in_=ots[s][:]
                )

            # prologue: both early groups' mains run back-to-back before
            # either group's tails — g1's mains fill the PE while g0's
            # (larger, later) tail transfer is still in flight
            emit_mains(0)
            emit_mains(1)
            emit_tails(0)
            emit_tails(1)
            # one-off fence ticks pre-satisfying group 2's cross-queue waits
            for dep_to, why in (
                (xmdmas[2], "x2 main"),
                (greds[0], "g0 reduces"),
                (gmms[0][14], "g0 psum done"),
            ):
                tk = _mm(
                    None,
                    pescr[0:2, 0:2],
                    junk[:, :, :2],
                    junk[:, :, :2],
                    start=True,
                    stop=True,
                    perf_mode=_DR,
                )
                _dep(tk, dep_to, f"tick {why}")

            for g in range(2, NG):
                if g >= 3:
                    # pre-satisfy group g's cross-queue waits on spare
                    # (waitless) matmuls of group g-1, so this group's
                    # branch-start matmuls carry no >1-wait encodings:
                    #   x DMAs done, g-2's reduces done (frees PSUM banks,
                    #   DVE sem), g-2's stop-matmul completed (PE sem).
                    prev = gmms[g - 1]
                    _dep(prev[4], xmdmas[g], "x main presat")
                    if xtdmas[g] is not None:
                        _dep(prev[5], xtdmas[g], "x tail presat")
                    _dep(prev[8], greds[g - 2], "psum reduce presat")
                    _dep(prev[9], gmms[g - 2][14], "psum group presat")

                # FC block s two groups after its features complete, then
                # its softmax slice two groups later still
                if g >= 5 and (g - 5) % 4 == 0:
                    _fc((g - 5) // 4)
                    smx_exp((g - 5) // 4)
                if g >= 7 and (g - 7) % 4 == 0:
                    smx_out((g - 7) // 4)

                # last group branch-sequential: k3/k4's reduce+relu overlap
                # the remaining conv matmuls instead of serializing on the
                # DVE after the final matmul
                seq = g == NG - 1
                emit_mains(g, seq=seq)
                if g == 2:
                    _dep(gmms[2][1], auxfc_dma, "FC weights presat")
                if not seq:
                    emit_tails(g)

            _fc(3)
            smx_exp(3)
            smx_out(3)
    return nc


def _prep(x, w3, b3, w4, b4, w5, b5, Wfc, bfc):
    x = np.asarray(x, dtype=np.float32).reshape(B, S, E)
    ws = {3: np.asarray(w3, np.float32)[:, 0], 4: np.asarray(w4, np.float32)[:, 0],
          5: np.asarray(w5, np.float32)[:, 0]}  # [NF, k, E]

    # weights region (identical across cores), assembled fp32 then cast once
    wreg = np.zeros((128, NW, 2, NFP), np.float32)
    for t, (k, i, _off) in enumerate(MAINTAPS):
        for j in range(2):
            wreg[:, t, j, :NF] = WS * ws[k][:, i, j * 128 : (j + 1) * 128].T
    for r, k in enumerate(KS):
        L = np.arange(E2N * k)
        i_of = L // E2N
        e_of = E2 + (L % E2N)
        wt = ws[k][:, i_of, e_of].T * WS  # [L, NF]
        wreg[L // 2, NTAP + r, L % 2, :NF] = wt
    wreg = wreg.reshape(128, WCOLS)

    # x padded + transposed: [E, B, SP]
    xt_all = np.zeros((E, B, SP), np.float32)
    xt_all[:, :, PAD : PAD + S] = x.transpose(2, 0, 1)

    auxm = np.zeros((NF + 1, AUXW), np.float32)
    for kk, bb in enumerate((b3, b4, b5)):
        auxm[:NF, kk] = np.asarray(bb, np.float32)
    Wfc = np.asarray(Wfc, np.float32)
    auxf = np.zeros((NF + 1, 3 * NCLS), np.float32)
    for kk in range(3):
        auxf[:NF, NCLS * kk : NCLS * (kk + 1)] = Wfc[
            :, kk * NF : (kk + 1) * NF
        ].T
    auxf[NF, 2 * NCLS : 3 * NCLS] = np.asarray(bfc, np.float32)
    auxf = auxf.astype(ml_dtypes.bfloat16)

    shards = []
    for c in range(NCORES):
        arr = np.zeros((128, TOTC), np.float32)
        arr[:, :WCOLS] = wreg
        xs = xt_all[:, c * BPC : (c + 1) * BPC, :]  # [E, 64, SP]
        for g in range(NG):
            xb = xs[:, g * G : (g + 1) * G, :]  # [E, G, SP]
            # s-major planes: col n = s*G + b, so shift o = slice [4o:4o+512]
            xbT = xb.transpose(0, 2, 1).reshape(E, SPW)  # [E, 528]
            sh = np.stack(
                [xb[:, :, o : o + S].transpose(0, 2, 1).reshape(E, NMM)
                 for o in range(5)]
            )  # [5, E, 512]
            blk = np.zeros((128, 2, SPW + 3 * NMM), np.float32)
            blk[:, 0, :SPW] = xbT[0:128]
            blk[:, 1, :SPW] = xbT[128:256]
            for r, k in enumerate(KS):  # packed tails, shifts baked
                L = np.arange(E2N * k)
                i_of = L // E2N
                e_of = E2 + (L % E2N)
                off = (5 - k) + i_of
                blk[L // 2, L % 2, SPW + r * NMM : SPW + (r + 1) * NMM] = sh[
                    off, e_of
                ]
            base = WCOLS + g * GCOLS
            arr[:, base : base + 2 * SPW] = blk[:, :, :SPW].reshape(128, 2 * SPW)
            arr[:, base + 2 * SPW : base + GCOLS] = blk[:, :, SPW:].reshape(
                128, 6 * NMM
            )
            if g == NG - 1:
                # b-major duplicate of the last group: batch-contiguous
                # blocks per shift, so its reduces read PSUM contiguously
                shb = np.stack(
                    [xb[:, :, o : o + S] for o in range(5)]
                )  # [5, E, G, S]
                bb = np.zeros((128, 8, 2, G, S), np.float32)
                for o in range(5):
                    bb[:, o, 0] = shb[o, 0:128]
                    bb[:, o, 1] = shb[o, 128:256]
                for r, k in enumerate(KS):
                    L = np.arange(E2N * k)
                    i_of = L // E2N
                    e_of = E2 + (L % E2N)
                    off = (5 - k) + i_of
                    bb[L // 2, 5 + r, L % 2] = shb[off, e_of]
                arr[:, TOTB:TOTC] = bb.reshape(128, BCOLS)
        shards.append(arr.astype(ml_dtypes.float8_e4m3))
    return shards, auxm, auxf


def _run(inputs, **spmd_kwargs):
    global _built
    if _built is None:
        _built = _build()
    shards, auxm, auxf = _prep(**inputs)
    in_maps = [
        {"xq": shards[c], "aux": auxm, "auxfc": auxf} for c in range(NCORES)
    ]
    res = run_bass_kernel_spmd(_built, in_maps, list(range(NCORES)), **spmd_kwargs)
    outp = np.concatenate(
        [np.asarray(res.results[c]["out"]) for c in range(NCORES)], axis=0
    )
    outp = np.negative(outp
    )
    return outp, res


def kernel(**inputs):
    outp, _ = _run(inputs)
    return outp
